# revision 35
# baseline (speedup 1.0000x reference)
"""MoE routing kernel (Autoformer-style gate + expert mix) on 8 TRN2 cores.

Data-parallel over batch: 8 samples/core. Key algebra: trend/seasonality are
only ever used through the w_start projection of new_x, so we never
materialize them at [B,L,N]. Per batch we compute the real DFT (f=1..95 plus
DC/Nyquist) of x as fp16 matmuls in [n,f] layout, rank |coef|^2 in fp32 to get
the top-3 mask, and project masked/unmasked coefs with w_start on the PE.
gx = Gall @ [masked_coefs; full_coefs] (Gall folds (I+T)∘IRDFT and the cosine
reconstruction), logits/gates on-chip, expert mix as one combined [192,192]
matmul per batch (weighted expert sum built on the PE from scaled identities,
bias folded in as a 193rd contraction row). Balance loss is computed on host
from the gathered gates.
"""
import sys

sys.path.insert(0, "/opt/trn_rl_repo")

from contextlib import ExitStack

import numpy as np

import concourse.bass as bass
import concourse.bacc as bacc
import concourse.tile as tile
from concourse import mybir
from concourse.bass_utils import run_bass_kernel_spmd

dt = mybir.dt
AF = mybir.ActivationFunctionType
ALU = mybir.AluOpType
AX = mybir.AxisListType

B, L, N, C = 64, 192, 862, 1
E, K = 4, 2
NCORES = 8
BLOC = B // NCORES          # 8 samples per core
F = 95                       # kept freqs 1..95
NCH = 7                      # ceil(862/128) n-chunks
KERNELS = (4, 8, 12)
BIG = 1e30

LAST_EXEC_NS = None


def _trend_matrix():
    I = np.eye(L)
    mats = []
    for k in KERNELS:
        front = np.repeat(I[:1], (k - 1) // 2, axis=0)
        end = np.repeat(I[-1:], k // 2, axis=0)
        ip = np.concatenate([front, I, end], axis=0)
        cs = np.cumsum(ip, axis=0)
        cs = np.concatenate([np.zeros((1, L)), cs], axis=0)
        mats.append((cs[k:] - cs[:-k]) / k)
    return sum(mats) / len(mats)            # [L(t), L(l)]


def _host_consts(w_start, b_start, w_gate, W_exp, b_exp, W_end, b_end):
    t = np.arange(L, dtype=np.float64)
    f = np.arange(1, F + 1, dtype=np.float64) / L
    ang = 2 * np.pi * np.outer(t, f)                      # [L, F]
    cosb, sinb = np.cos(ang), np.sin(ang)
    # FFT basis columns: [cos 95 | sin 95 | DC | Nyquist]
    basis = np.concatenate(
        [cosb, sinb, np.ones((L, 1)), ((-1.0) ** t)[:, None]], axis=1
    )                                                      # [L, 192]
    basis16 = basis.astype(np.float16)

    # inverse real DFT:  s = IR @ cfull  (cfull ordered like basis columns)
    IR = np.concatenate(
        [2.0 * cosb, 2.0 * sinb, np.ones((L, 1)), ((-1.0) ** t)[:, None]], axis=1
    ) / L                                                  # [L, 192]
    T = _trend_matrix()
    G1 = (np.eye(L) + T) @ IR                              # [L, 192] full-coef map
    G2 = (2.0 / L) * np.concatenate([cosb, sinb], axis=1)  # [L, 190] masked map
    Gall = np.concatenate([G2, G1], axis=1)                # [L, 382]
    GallT = np.ascontiguousarray(Gall.T.astype(np.float32))  # [382, L]

    # w_start per n-chunk as fp16 hi/lo pairs: [128, 2*NCH]
    wpad = np.zeros(NCH * 128, np.float64)
    wpad[:N] = w_start.astype(np.float64)
    wsb = np.zeros((128, 2 * NCH), np.float16)
    for c in range(NCH):
        col = wpad[c * 128:(c + 1) * 128]
        hi = col.astype(np.float16)
        lo = (col - hi.astype(np.float64)).astype(np.float16)
        wsb[:, 2 * c] = hi
        wsb[:, 2 * c + 1] = lo

    wg = w_gate.astype(np.float32)                         # [L, E]
    biasE = (float(b_start) * wg.sum(axis=0)).astype(np.float32)  # [E]
    biasE8 = np.tile(biasE[None, :], (BLOC, 1))            # [8, 4]

    # expert stack [5, 193, 192] fp16: rows 0..191 = W (l,m), row 192 = bias
    wstack = np.zeros((E + 1, L + 1, L), np.float16)
    for e in range(E):
        wstack[e, :L] = W_exp[e].astype(np.float16)
        wstack[e, L] = b_exp[e].astype(np.float16)
    wstack[E, :L] = W_end.T.astype(np.float16)
    wstack[E, L] = b_end.astype(np.float16)

    eye16 = np.eye(128, dtype=np.float16)
    eyef = np.eye(128, dtype=np.float32)
    ones1 = np.ones((1, 128), np.float32)
    return dict(basis=basis16, gallt=GallT, wsb=wsb, wg=wg, biasE=biasE8,
                wstack=wstack, eye16=eye16, eyef=eyef, ones1=ones1)


def _build_graph():
    nc = bacc.Bacc()
    f16, f32 = dt.float16, dt.float32
    xq = nc.declare_dram_parameter("x16", [BLOC, L + 1, N], f16, isOutput=False)
    basis_d = nc.declare_dram_parameter("basis", [L, 192], f16, isOutput=False)
    gallt_d = nc.declare_dram_parameter("gallt", [382, L], f32, isOutput=False)
    wsb_d = nc.declare_dram_parameter("wsb", [128, 2 * NCH], f16, isOutput=False)
    wg_d = nc.declare_dram_parameter("wg", [L, E], f32, isOutput=False)
    biasE_d = nc.declare_dram_parameter("biasE", [BLOC, E], f32, isOutput=False)
    wstack_d = nc.declare_dram_parameter("wstack", [E + 1, L + 1, L], f16,
                                         isOutput=False)
    eye16_d = nc.declare_dram_parameter("eye16", [128, 128], f16, isOutput=False)
    eyef_d = nc.declare_dram_parameter("eyef", [128, 128], f32, isOutput=False)
    ones1_d = nc.declare_dram_parameter("ones1", [1, 128], f32, isOutput=False)
    out_d = nc.declare_dram_parameter("out", [BLOC, L, N], f32, isOutput=True)
    gates_d = nc.declare_dram_parameter("gates", [BLOC, E], f32, isOutput=True)
    gsc = nc.dram_tensor("gsc", [1, BLOC * E], f32)
    cpd = nc.dram_tensor("cpd", [2 * BLOC, 382], f32)

    with tile.TileContext(nc) as tc, ExitStack() as ctx:
        cst = ctx.enter_context(tc.tile_pool(name="cst", bufs=1))
        xpool = ctx.enter_context(tc.tile_pool(name="xp", bufs=1))
        work = ctx.enter_context(tc.tile_pool(name="wk", bufs=6))
        small = ctx.enter_context(tc.tile_pool(name="sm", bufs=4))
        csp = ctx.enter_context(tc.tile_pool(name="csp", bufs=3, space="PSUM"))
        prp = ctx.enter_context(tc.tile_pool(name="prp", bufs=1, space="PSUM"))
        pbc = ctx.enter_context(tc.tile_pool(name="pbc", bufs=2, space="PSUM"))
        outp = ctx.enter_context(tc.tile_pool(name="outp", bufs=2, space="PSUM"))

        def cload(name, dram, shape, dtp, tag):
            tl = cst.tile(shape, dtp, tag=tag)
            nc.gpsimd.dma_start(tl[:], dram)
            return tl

        bas1 = cload("b1", basis_d[0:128, :], [128, 192], f16, "bas1")
        bas2 = cload("b2", basis_d[128:L, :], [64, 192], f16, "bas2")
        ga1 = cload("g1", gallt_d[0:128, :], [128, L], f32, "ga1")
        ga2 = cload("g2", gallt_d[128:256, :], [128, L], f32, "ga2")
        ga3 = cload("g3", gallt_d[256:382, :], [126, L], f32, "ga3")
        wsb = cload("w", wsb_d[:, :], [128, 2 * NCH], f16, "wsb")
        wg1 = cload("wga", wg_d[0:128, :], [128, E], f32, "wg1")
        wg2 = cload("wgb", wg_d[128:L, :], [64, E], f32, "wg2")
        biasE = cload("be", biasE_d[:, :], [BLOC, E], f32, "biasE")
        eye16 = cload("e16", eye16_d[:, :], [128, 128], f16, "eye16")
        eyef = cload("ef", eyef_d[:, :], [128, 128], f32, "eyef")
        ones1 = cload("o1", ones1_d[:, :], [1, 128], f32, "ones1")
        wstA, wstB = [], []
        for e in range(E + 1):
            wstA.append(cload(f"wa{e}", wstack_d[e, 0:128, :], [128, L], f16,
                              f"wa{e}"))
            wstB.append(cload(f"wb{e}", wstack_d[e, 128:L + 1, :], [65, L], f16,
                              f"wb{e}"))

        cPall = cst.tile([2 * BLOC, 382], f32, tag="cPall")
        xt = []
        for b in range(BLOC):
            x1 = xpool.tile([128, N], f16, tag=f"x1_{b}")
            nc.gpsimd.dma_start(x1[:], xq[b, 0:128, :])
            x2 = xpool.tile([65, N], f16, tag=f"x2_{b}")
            nc.gpsimd.dma_start(x2[:], xq[b, 128:L + 1, :])
            xt.append((x1, x2))

        # ---------------- phase A: FFT + top-3 mask + projections ----------
        for b in range(BLOC):
            x1, x2 = xt[b]
            pp = prp.tile([2, 382], f32, tag="pp")
            magAB = work.tile([128, NCH * 2 * F], f32, tag="magAB")
            prB = work.tile([128, NCH * 382], f16, tag="prB")
            for c in range(NCH):
                n0 = c * 128
                w = min(128, N - n0)
                cs = csp.tile([128, 192], f32, tag="cs")
                nc.tensor.matmul(cs[:w, :], x1[:, n0:n0 + w], bas1[:],
                                 start=True, stop=False)
                nc.tensor.matmul(cs[:w, :], x2[0:64, n0:n0 + w], bas2[:],
                                 start=False, stop=True)
                nc.scalar.activation(magAB[:w, 190 * c:190 * (c + 1)],
                                     cs[:w, 0:2 * F], AF.Square)
                nc.scalar.activation(prB[:w, 382 * c + 190:382 * (c + 1)],
                                     cs[:w, 0:192], AF.Copy)
            # batched top-3 mask over all 7 chunks at once
            mav = magAB[:, :].rearrange("p (c f) -> p c f", c=NCH)
            mag = work.tile([128, NCH * F], f32, tag="mag")
            magv = mag[:, :].rearrange("p (c f) -> p c f", c=NCH)
            nc.vector.tensor_add(magv, mav[:, :, 0:F], mav[:, :, F:2 * F])
            r = small.tile([128, 3 * NCH], f32, tag="r")
            m2 = work.tile([128, NCH * F], f32, tag="m2")
            m2v = m2[:, :].rearrange("p (c f) -> p c f", c=NCH)
            excl = work.tile([128, NCH * F], f32, tag="excl")
            ev = excl[:, :].rearrange("p (c f) -> p c f", c=NCH)
            nc.vector.tensor_reduce(r[:, 0:NCH], magv, AX.X, ALU.max)
            r1b = r[:, 0:NCH].unsqueeze(2).broadcast_to((128, NCH, F))
            nc.vector.tensor_tensor(ev, magv, r1b, ALU.is_ge)
            nc.vector.scalar_tensor_tensor(m2v, ev, -BIG, magv,
                                           ALU.mult, ALU.add)
            nc.vector.tensor_reduce(r[:, NCH:2 * NCH], m2v, AX.X, ALU.max)
            r2b = r[:, NCH:2 * NCH].unsqueeze(2).broadcast_to((128, NCH, F))
            nc.vector.tensor_tensor(ev, m2v, r2b, ALU.is_ge)
            nc.vector.scalar_tensor_tensor(m2v, ev, -BIG, m2v,
                                           ALU.mult, ALU.add)
            nc.vector.tensor_reduce(r[:, 2 * NCH:3 * NCH], m2v, AX.X, ALU.max)
            thr = small.tile([128, NCH], f32, tag="thr")
            nc.vector.tensor_scalar_mul(thr[:, :], r[:, 2 * NCH:3 * NCH],
                                        1.0 - 1e-6)
            selB = work.tile([128, NCH * F], f16, tag="selB")
            selv = selB[:, :].rearrange("p (c f) -> p c f", c=NCH)
            thrb = thr[:, :].unsqueeze(2).broadcast_to((128, NCH, F))
            nc.vector.tensor_tensor(selv, magv, thrb, ALU.is_ge)
            prv = prB[:, :].rearrange("p (c f) -> p c f", c=NCH)
            nc.vector.tensor_mul(prv[:, :, 0:F], selv, prv[:, :, 190:285])
            nc.vector.tensor_mul(prv[:, :, F:2 * F], selv, prv[:, :, 285:380])
            for c in range(NCH):
                w = min(128, N - c * 128)
                nc.tensor.matmul(pp[:, :], wsb[:w, 2 * c:2 * c + 2],
                                 prB[:w, 382 * c:382 * (c + 1)],
                                 start=(c == 0), stop=(c == NCH - 1))
            cPb = small.tile([2, 382], f32, tag="cPb")
            nc.scalar.activation(cPb[:, :], pp[:, :], AF.Copy)
            nc.sync.dma_start(cpd[2 * b:2 * b + 2, :], cPb[:, :])

        # ---------------- phase B: gx -> logits -> gates -------------------
        nc.gpsimd.dma_start(cPall[:, :], cpd[:, :])
        ctp = []
        for i, (c0, cw) in enumerate(((0, 128), (128, 128), (256, 126))):
            tp = pbc.tile([128, L], f32, tag="pb")
            nc.tensor.transpose(tp[:cw, 0:2 * BLOC], cPall[:, c0:c0 + cw],
                                eyef[0:2 * BLOC, 0:2 * BLOC])
            ts_ = work.tile([128, 2 * BLOC], f32, tag=f"cts{i}")
            nc.scalar.activation(ts_[:cw, :], tp[:cw, 0:2 * BLOC], AF.Copy)
            tm = work.tile([128, BLOC], f32, tag=f"ctm{i}")
            nc.vector.tensor_add(tm[:cw, :], ts_[:cw, 0:2 * BLOC:2],
                                 ts_[:cw, 1:2 * BLOC:2])
            ctp.append((tm, cw))
        gxp1 = pbc.tile([128, L], f32, tag="pb")
        gxp2 = pbc.tile([128, L], f32, tag="pb")
        gas = ((ga1, 128), (ga2, 128), (ga3, 126))
        for i, ((tsm, cw), (ga, _)) in enumerate(zip(ctp, gas)):
            nc.tensor.matmul(gxp1[:, 0:BLOC], ga[:cw, 0:128], tsm[:cw, :],
                             start=(i == 0), stop=(i == 2))
        for i, ((tsm, cw), (ga, _)) in enumerate(zip(ctp, gas)):
            nc.tensor.matmul(gxp2[0:64, 0:BLOC], ga[:cw, 128:L], tsm[:cw, :],
                             start=(i == 0), stop=(i == 2))
        gxs1 = work.tile([128, BLOC], f32, tag="gxs1")
        gxs2 = work.tile([64, BLOC], f32, tag="gxs2")
        nc.scalar.activation(gxs1[:, :], gxp1[:, 0:BLOC], AF.Copy)
        nc.scalar.activation(gxs2[:, :], gxp2[0:64, 0:BLOC], AF.Copy)
        lgp = pbc.tile([128, L], f32, tag="pb")
        nc.tensor.matmul(lgp[0:BLOC, 0:E], gxs1[:, :], wg1[:], start=True, stop=False)
        nc.tensor.matmul(lgp[0:BLOC, 0:E], gxs2[:, :], wg2[:], start=False, stop=True)
        lgs = small.tile([BLOC, E], f32, tag="lgs")
        nc.scalar.activation(lgs[:, :], lgp[0:BLOC, 0:E], AF.Copy)
        lg = small.tile([BLOC, E], f32, tag="lg")
        nc.vector.tensor_add(lg[:, :], lgs[:, :], biasE[:, :])
        # top-2 softmax gates
        q = small.tile([BLOC, 2], f32, tag="q")
        nc.vector.tensor_reduce(q[:, 0:1], lg[:, :], AX.X, ALU.max)
        e1 = small.tile([BLOC, E], f32, tag="e1")
        nc.vector.tensor_scalar(e1[:, :], lg[:, :], q[:, 0:1], None, ALU.is_ge)
        lx = small.tile([BLOC, E], f32, tag="lx")
        nc.vector.tensor_scalar(lx[:, :], lg[:, :], q[:, 0:1], -BIG,
                                ALU.is_ge, ALU.mult)
        nc.vector.tensor_add(lx[:, :], lg[:, :], lx[:, :])
        nc.vector.tensor_reduce(q[:, 1:2], lx[:, :], AX.X, ALU.max)
        e2 = small.tile([BLOC, E], f32, tag="e2")
        nc.vector.tensor_scalar(e2[:, :], lx[:, :], q[:, 1:2], None, ALU.is_ge)
        w2 = small.tile([BLOC, 2], f32, tag="w2")
        nc.scalar.activation(w2[:, 0:1], q[:, 1:2], AF.Sigmoid,
                             bias=q[:, 0:1], scale=-1.0)  # sigmoid(m1*-1+... )
        # careful: want sigmoid(m2 - m1): in_=q[:,1:2], scale=1, bias=-m1
        gates = small.tile([BLOC, E], f32, tag="gates")
        w1 = small.tile([BLOC, 2], f32, tag="w1")
        nc.vector.tensor_scalar(w1[:, 0:1], w2[:, 0:1], -1.0, 1.0,
                                ALU.mult, ALU.add)
        # w2 tile holds sigmoid(m1-m2) = weight of the argmax (top-1)
        g1t = small.tile([BLOC, E], f32, tag="g1t")
        nc.vector.tensor_scalar_mul(g1t[:, :], e1[:, :], w2[:, 0:1])
        g2t = small.tile([BLOC, E], f32, tag="g2t")
        nc.vector.tensor_scalar_mul(g2t[:, :], e2[:, :], w1[:, 0:1])
        nc.vector.tensor_add(gates[:, :], g1t[:, :], g2t[:, :])
        nc.sync.dma_start(gates_d[:, :], gates[:, :])
        # broadcast gates to all partitions: gates -> dram [1,32] -> matmul
        nc.sync.dma_start(gsc[:, :], gates[:, :])
        g1x = small.tile([1, BLOC * E], f32, tag="g1x")
        nc.gpsimd.dma_start(g1x[:, :], gsc[:, :])
        gbp = pbc.tile([128, L], f32, tag="pb")
        nc.tensor.matmul(gbp[:, 0:BLOC * E], ones1[:], g1x[:, :], start=True, stop=True)
        gbc = work.tile([128, BLOC * E], f32, tag="gbc")
        nc.scalar.activation(gbc[:, :], gbp[:, 0:BLOC * E], AF.Copy)

        # ---------------- phase C: combined expert matmul ------------------
        for b in range(BLOC):
            x1, x2 = xt[b]
            cA = pbc.tile([128, L], f32, tag="pb")
            cB = pbc.tile([128, L], f32, tag="pb")
            gis = []
            for e in range(E):
                gi = work.tile([128, 128], f16, tag=f"gi{e}")
                nc.vector.tensor_scalar_mul(gi[:, :], eye16[:, :],
                                            gbc[:, E * b + e:E * b + e + 1])
                gis.append(gi)
            gis.append(eye16)
            for e in range(E + 1):
                nc.tensor.matmul(cA[:, :], gis[e][:, :], wstA[e][:],
                                 start=(e == 0), stop=(e == E))
            for e in range(E + 1):
                nc.tensor.matmul(cB[0:65, :], gis[e][0:65, 0:65], wstB[e][:],
                                 start=(e == 0), stop=(e == E))
            c16A = work.tile([128, L], f16, tag="c16A")
            c16B = work.tile([65, L], f16, tag="c16B")
            nc.scalar.activation(c16A[:, :], cA[:, :], AF.Copy)
            nc.scalar.activation(c16B[:, :], cB[0:65, :], AF.Copy)
            for mi, (m0, mw) in enumerate(((0, 128), (128, 64))):
                for ni, (nn0, nw) in enumerate(((0, 431), (431, 431))):
                    op = outp.tile([128, 431], f32, tag="op")
                    nc.tensor.matmul(op[:mw, :nw], c16A[:, m0:m0 + mw],
                                     x1[:, nn0:nn0 + nw], start=True, stop=False)
                    nc.tensor.matmul(op[:mw, :nw], c16B[:, m0:m0 + mw],
                                     x2[:, nn0:nn0 + nw], start=False, stop=True)
                    osb = work.tile([128, 431], f32, tag="osb")
                    nc.scalar.activation(osb[:mw, :nw], op[:mw, :nw], AF.Copy)
                    nc.sync.dma_start(out_d[b, m0:m0 + mw, nn0:nn0 + nw],
                                      osb[:mw, :nw])
    nc.finalize()
    return nc


def kernel(x, w_start, b_start, w_gate, W_exp, b_exp, W_end, b_end):
    global LAST_EXEC_NS
    x = np.asarray(x)
    consts = _host_consts(np.asarray(w_start), np.asarray(b_start),
                          np.asarray(w_gate), np.asarray(W_exp),
                          np.asarray(b_exp), np.asarray(W_end),
                          np.asarray(b_end))
    x3 = x[:, :, :, 0]
    in_maps = []
    for i in range(NCORES):
        xs = x3[i * BLOC:(i + 1) * BLOC]            # [8, L, N]
        x16 = np.ones((BLOC, L + 1, N), np.float16)
        x16[:, :L, :] = xs.astype(np.float16)
        m = {"x16": x16}
        m.update(consts)
        in_maps.append(m)
    nc = _build_graph()
    import os
    tr = os.environ.get("BASS_DO_TRACE", "1") == "1"
    try:
        res = run_bass_kernel_spmd(nc, in_maps, list(range(NCORES)), trace=tr)
    except Exception:
        if not tr:
            raise
        res = run_bass_kernel_spmd(nc, in_maps, list(range(NCORES)))
    LAST_EXEC_NS = res.exec_time_ns
    outs = np.concatenate([r["out"] for r in res.results], axis=0)
    gates = np.concatenate([r["gates"] for r in res.results], axis=0)
    # balance loss on host (fp32, mirrors reference)
    imp = gates.sum(axis=0)
    load = (gates > 0).sum(axis=0).astype(np.float32)

    def cv2(v):
        return v.var(ddof=1) / (v.mean() ** 2 + 1e-10)

    loss = np.float32(0.01 * (cv2(imp) + cv2(load)))
    return outs.reshape(B, L, N, C).astype(np.float32), loss


# revision 36
# speedup vs baseline: 1.0005x; 1.0005x over previous
"""MoE routing kernel (Autoformer-style gate + expert mix) on 8 TRN2 cores.

Data-parallel over batch: 8 samples/core. Key algebra: trend/seasonality are
only ever used through the w_start projection of new_x, so we never
materialize them at [B,L,N]. Per batch we compute the real DFT (f=1..95 plus
DC/Nyquist) of x as fp16 matmuls in [n,f] layout, rank |coef|^2 in fp32 to get
the top-3 mask, and project masked/unmasked coefs with w_start on the PE.
gx = Gall @ [masked_coefs; full_coefs] (Gall folds (I+T)∘IRDFT and the cosine
reconstruction), logits/gates on-chip, expert mix as one combined [192,192]
matmul per batch (weighted expert sum built on the PE from scaled identities,
bias folded in as a 193rd contraction row). Balance loss is computed on host
from the gathered gates.
"""
import sys

sys.path.insert(0, "/opt/trn_rl_repo")

from contextlib import ExitStack

import numpy as np

import concourse.bass as bass
import concourse.bacc as bacc
import concourse.tile as tile
from concourse import mybir
from concourse.bass_utils import run_bass_kernel_spmd

dt = mybir.dt
AF = mybir.ActivationFunctionType
ALU = mybir.AluOpType
AX = mybir.AxisListType

B, L, N, C = 64, 192, 862, 1
E, K = 4, 2
NCORES = 8
BLOC = B // NCORES          # 8 samples per core
F = 95                       # kept freqs 1..95
NCH = 7                      # ceil(862/128) n-chunks
KERNELS = (4, 8, 12)
BIG = 1e30

LAST_EXEC_NS = None


def _trend_matrix():
    I = np.eye(L)
    mats = []
    for k in KERNELS:
        front = np.repeat(I[:1], (k - 1) // 2, axis=0)
        end = np.repeat(I[-1:], k // 2, axis=0)
        ip = np.concatenate([front, I, end], axis=0)
        cs = np.cumsum(ip, axis=0)
        cs = np.concatenate([np.zeros((1, L)), cs], axis=0)
        mats.append((cs[k:] - cs[:-k]) / k)
    return sum(mats) / len(mats)            # [L(t), L(l)]


def _host_consts(w_start, b_start, w_gate, W_exp, b_exp, W_end, b_end):
    t = np.arange(L, dtype=np.float64)
    f = np.arange(1, F + 1, dtype=np.float64) / L
    ang = 2 * np.pi * np.outer(t, f)                      # [L, F]
    cosb, sinb = np.cos(ang), np.sin(ang)
    # FFT basis columns: [cos 95 | sin 95 | DC | Nyquist]
    basis = np.concatenate(
        [cosb, sinb, np.ones((L, 1)), ((-1.0) ** t)[:, None]], axis=1
    )                                                      # [L, 192]
    basis16 = basis.astype(np.float16)

    # inverse real DFT:  s = IR @ cfull  (cfull ordered like basis columns)
    IR = np.concatenate(
        [2.0 * cosb, 2.0 * sinb, np.ones((L, 1)), ((-1.0) ** t)[:, None]], axis=1
    ) / L                                                  # [L, 192]
    T = _trend_matrix()
    G1 = (np.eye(L) + T) @ IR                              # [L, 192] full-coef map
    G2 = (2.0 / L) * np.concatenate([cosb, sinb], axis=1)  # [L, 190] masked map
    Gall = np.concatenate([G2, G1], axis=1)                # [L, 382]
    GallT = np.ascontiguousarray(Gall.T.astype(np.float32))  # [382, L]

    # w_start per n-chunk as fp16 hi/lo pairs: [128, 2*NCH]
    wpad = np.zeros(NCH * 128, np.float64)
    wpad[:N] = w_start.astype(np.float64)
    wsb = np.zeros((128, 2 * NCH), np.float16)
    for c in range(NCH):
        col = wpad[c * 128:(c + 1) * 128]
        hi = col.astype(np.float16)
        lo = (col - hi.astype(np.float64)).astype(np.float16)
        wsb[:, 2 * c] = hi
        wsb[:, 2 * c + 1] = lo

    wg = w_gate.astype(np.float32)                         # [L, E]
    biasE = (float(b_start) * wg.sum(axis=0)).astype(np.float32)  # [E]
    biasE8 = np.tile(biasE[None, :], (BLOC, 1))            # [8, 4]

    # expert stack [5, 193, 192] fp16: rows 0..191 = W (l,m), row 192 = bias
    wstack = np.zeros((E + 1, L + 1, L), np.float16)
    for e in range(E):
        wstack[e, :L] = W_exp[e].astype(np.float16)
        wstack[e, L] = b_exp[e].astype(np.float16)
    wstack[E, :L] = W_end.T.astype(np.float16)
    wstack[E, L] = b_end.astype(np.float16)

    eye16 = np.eye(128, dtype=np.float16)
    eyef = np.eye(128, dtype=np.float32)
    ones1 = np.ones((1, 128), np.float32)
    return dict(basis=basis16, gallt=GallT, wsb=wsb, wg=wg, biasE=biasE8,
                wstack=wstack, eye16=eye16, eyef=eyef, ones1=ones1)


def _build_graph():
    nc = bacc.Bacc()
    f16, f32 = dt.float16, dt.float32
    xq = nc.declare_dram_parameter("x16", [BLOC, L + 1, N], f16, isOutput=False)
    basis_d = nc.declare_dram_parameter("basis", [L, 192], f16, isOutput=False)
    gallt_d = nc.declare_dram_parameter("gallt", [382, L], f32, isOutput=False)
    wsb_d = nc.declare_dram_parameter("wsb", [128, 2 * NCH], f16, isOutput=False)
    wg_d = nc.declare_dram_parameter("wg", [L, E], f32, isOutput=False)
    biasE_d = nc.declare_dram_parameter("biasE", [BLOC, E], f32, isOutput=False)
    wstack_d = nc.declare_dram_parameter("wstack", [E + 1, L + 1, L], f16,
                                         isOutput=False)
    eye16_d = nc.declare_dram_parameter("eye16", [128, 128], f16, isOutput=False)
    eyef_d = nc.declare_dram_parameter("eyef", [128, 128], f32, isOutput=False)
    ones1_d = nc.declare_dram_parameter("ones1", [1, 128], f32, isOutput=False)
    out_d = nc.declare_dram_parameter("out", [BLOC, L, N], f32, isOutput=True)
    gates_d = nc.declare_dram_parameter("gates", [BLOC, E], f32, isOutput=True)
    gsc = nc.dram_tensor("gsc", [1, BLOC * E], f32)
    cpd = nc.dram_tensor("cpd", [2 * BLOC, 382], f32)

    with tile.TileContext(nc) as tc, ExitStack() as ctx:
        cst = ctx.enter_context(tc.tile_pool(name="cst", bufs=1))
        xpool = ctx.enter_context(tc.tile_pool(name="xp", bufs=1))
        work = ctx.enter_context(tc.tile_pool(name="wk", bufs=6))
        small = ctx.enter_context(tc.tile_pool(name="sm", bufs=8))
        csp = ctx.enter_context(tc.tile_pool(name="csp", bufs=3, space="PSUM"))
        prp = ctx.enter_context(tc.tile_pool(name="prp", bufs=1, space="PSUM"))
        pbc = ctx.enter_context(tc.tile_pool(name="pbc", bufs=2, space="PSUM"))
        outp = ctx.enter_context(tc.tile_pool(name="outp", bufs=2, space="PSUM"))

        def cload(name, dram, shape, dtp, tag):
            tl = cst.tile(shape, dtp, tag=tag)
            nc.gpsimd.dma_start(tl[:], dram)
            return tl

        bas1 = cload("b1", basis_d[0:128, :], [128, 192], f16, "bas1")
        bas2 = cload("b2", basis_d[128:L, :], [64, 192], f16, "bas2")
        ga1 = cload("g1", gallt_d[0:128, :], [128, L], f32, "ga1")
        ga2 = cload("g2", gallt_d[128:256, :], [128, L], f32, "ga2")
        ga3 = cload("g3", gallt_d[256:382, :], [126, L], f32, "ga3")
        wsb = cload("w", wsb_d[:, :], [128, 2 * NCH], f16, "wsb")
        wg1 = cload("wga", wg_d[0:128, :], [128, E], f32, "wg1")
        wg2 = cload("wgb", wg_d[128:L, :], [64, E], f32, "wg2")
        biasE = cload("be", biasE_d[:, :], [BLOC, E], f32, "biasE")
        eye16 = cload("e16", eye16_d[:, :], [128, 128], f16, "eye16")
        eyef = cload("ef", eyef_d[:, :], [128, 128], f32, "eyef")
        ones1 = cload("o1", ones1_d[:, :], [1, 128], f32, "ones1")
        wstA, wstB = [], []
        for e in range(E + 1):
            wstA.append(cload(f"wa{e}", wstack_d[e, 0:128, :], [128, L], f16,
                              f"wa{e}"))
            wstB.append(cload(f"wb{e}", wstack_d[e, 128:L + 1, :], [65, L], f16,
                              f"wb{e}"))

        cPall = cst.tile([2 * BLOC, 382], f32, tag="cPall")
        xt = []
        for b in range(BLOC):
            x1 = xpool.tile([128, N], f16, tag=f"x1_{b}")
            nc.gpsimd.dma_start(x1[:], xq[b, 0:128, :])
            x2 = xpool.tile([65, N], f16, tag=f"x2_{b}")
            nc.gpsimd.dma_start(x2[:], xq[b, 128:L + 1, :])
            xt.append((x1, x2))

        # ---------------- phase A: FFT + top-3 mask + projections ----------
        for b in range(BLOC):
            x1, x2 = xt[b]
            pp = prp.tile([2, 382], f32, tag="pp")
            magAB = work.tile([128, NCH * 2 * F], f32, tag="magAB")
            prB = work.tile([128, NCH * 382], f16, tag="prB")
            for c in range(NCH):
                n0 = c * 128
                w = min(128, N - n0)
                cs = csp.tile([128, 192], f32, tag="cs")
                nc.tensor.matmul(cs[:w, :], x1[:, n0:n0 + w], bas1[:],
                                 start=True, stop=False)
                nc.tensor.matmul(cs[:w, :], x2[0:64, n0:n0 + w], bas2[:],
                                 start=False, stop=True)
                nc.scalar.activation(magAB[:w, 190 * c:190 * (c + 1)],
                                     cs[:w, 0:2 * F], AF.Square)
                nc.scalar.activation(prB[:w, 382 * c + 190:382 * (c + 1)],
                                     cs[:w, 0:192], AF.Copy)
            # batched top-3 mask over all 7 chunks at once
            mav = magAB[:, :].rearrange("p (c f) -> p c f", c=NCH)
            mag = work.tile([128, NCH * F], f32, tag="mag")
            magv = mag[:, :].rearrange("p (c f) -> p c f", c=NCH)
            nc.vector.tensor_add(magv, mav[:, :, 0:F], mav[:, :, F:2 * F])
            r = small.tile([128, 3 * NCH], f32, tag="r")
            m2 = work.tile([128, NCH * F], f32, tag="m2")
            m2v = m2[:, :].rearrange("p (c f) -> p c f", c=NCH)
            excl = work.tile([128, NCH * F], f32, tag="excl")
            ev = excl[:, :].rearrange("p (c f) -> p c f", c=NCH)
            nc.vector.tensor_reduce(r[:, 0:NCH], magv, AX.X, ALU.max)
            r1b = r[:, 0:NCH].unsqueeze(2).broadcast_to((128, NCH, F))
            nc.vector.tensor_tensor(ev, magv, r1b, ALU.is_ge)
            nc.vector.scalar_tensor_tensor(m2v, ev, -BIG, magv,
                                           ALU.mult, ALU.add)
            nc.vector.tensor_reduce(r[:, NCH:2 * NCH], m2v, AX.X, ALU.max)
            r2b = r[:, NCH:2 * NCH].unsqueeze(2).broadcast_to((128, NCH, F))
            nc.vector.tensor_tensor(ev, m2v, r2b, ALU.is_ge)
            nc.vector.scalar_tensor_tensor(m2v, ev, -BIG, m2v,
                                           ALU.mult, ALU.add)
            nc.vector.tensor_reduce(r[:, 2 * NCH:3 * NCH], m2v, AX.X, ALU.max)
            thr = small.tile([128, NCH], f32, tag="thr")
            nc.vector.tensor_scalar_mul(thr[:, :], r[:, 2 * NCH:3 * NCH],
                                        1.0 - 1e-6)
            selB = work.tile([128, NCH * F], f16, tag="selB")
            selv = selB[:, :].rearrange("p (c f) -> p c f", c=NCH)
            thrb = thr[:, :].unsqueeze(2).broadcast_to((128, NCH, F))
            nc.vector.tensor_tensor(selv, magv, thrb, ALU.is_ge)
            prv = prB[:, :].rearrange("p (c f) -> p c f", c=NCH)
            nc.vector.tensor_mul(prv[:, :, 0:F], selv, prv[:, :, 190:285])
            nc.vector.tensor_mul(prv[:, :, F:2 * F], selv, prv[:, :, 285:380])
            for c in range(NCH):
                w = min(128, N - c * 128)
                nc.tensor.matmul(pp[:, :], wsb[:w, 2 * c:2 * c + 2],
                                 prB[:w, 382 * c:382 * (c + 1)],
                                 start=(c == 0), stop=(c == NCH - 1))
            cPb = small.tile([2, 382], f32, tag="cPb")
            nc.scalar.activation(cPb[:, :], pp[:, :], AF.Copy)
            nc.sync.dma_start(cpd[2 * b:2 * b + 2, :], cPb[:, :])

        # ---------------- phase B: gx -> logits -> gates -------------------
        nc.gpsimd.dma_start(cPall[:, :], cpd[:, :])
        ctp = []
        for i, (c0, cw) in enumerate(((0, 128), (128, 128), (256, 126))):
            tp = pbc.tile([128, L], f32, tag="pb")
            nc.tensor.transpose(tp[:cw, 0:2 * BLOC], cPall[:, c0:c0 + cw],
                                eyef[0:2 * BLOC, 0:2 * BLOC])
            ts_ = work.tile([128, 2 * BLOC], f32, tag=f"cts{i}")
            nc.scalar.activation(ts_[:cw, :], tp[:cw, 0:2 * BLOC], AF.Copy)
            tm = work.tile([128, BLOC], f32, tag=f"ctm{i}")
            nc.vector.tensor_add(tm[:cw, :], ts_[:cw, 0:2 * BLOC:2],
                                 ts_[:cw, 1:2 * BLOC:2])
            ctp.append((tm, cw))
        gxp1 = pbc.tile([128, L], f32, tag="pb")
        gxp2 = pbc.tile([128, L], f32, tag="pb")
        gas = ((ga1, 128), (ga2, 128), (ga3, 126))
        for i, ((tsm, cw), (ga, _)) in enumerate(zip(ctp, gas)):
            nc.tensor.matmul(gxp1[:, 0:BLOC], ga[:cw, 0:128], tsm[:cw, :],
                             start=(i == 0), stop=(i == 2))
        for i, ((tsm, cw), (ga, _)) in enumerate(zip(ctp, gas)):
            nc.tensor.matmul(gxp2[0:64, 0:BLOC], ga[:cw, 128:L], tsm[:cw, :],
                             start=(i == 0), stop=(i == 2))
        gxs1 = work.tile([128, BLOC], f32, tag="gxs1")
        gxs2 = work.tile([64, BLOC], f32, tag="gxs2")
        nc.scalar.activation(gxs1[:, :], gxp1[:, 0:BLOC], AF.Copy)
        nc.scalar.activation(gxs2[:, :], gxp2[0:64, 0:BLOC], AF.Copy)
        lgp = pbc.tile([128, L], f32, tag="pb")
        nc.tensor.matmul(lgp[0:BLOC, 0:E], gxs1[:, :], wg1[:], start=True, stop=False)
        nc.tensor.matmul(lgp[0:BLOC, 0:E], gxs2[:, :], wg2[:], start=False, stop=True)
        lgs = small.tile([BLOC, E], f32, tag="lgs")
        nc.scalar.activation(lgs[:, :], lgp[0:BLOC, 0:E], AF.Copy)
        lg = small.tile([BLOC, E], f32, tag="lg")
        nc.vector.tensor_add(lg[:, :], lgs[:, :], biasE[:, :])
        # top-2 softmax gates
        q = small.tile([BLOC, 2], f32, tag="q")
        nc.vector.tensor_reduce(q[:, 0:1], lg[:, :], AX.X, ALU.max)
        e1 = small.tile([BLOC, E], f32, tag="e1")
        nc.vector.tensor_scalar(e1[:, :], lg[:, :], q[:, 0:1], None, ALU.is_ge)
        lx = small.tile([BLOC, E], f32, tag="lx")
        nc.vector.tensor_scalar(lx[:, :], lg[:, :], q[:, 0:1], -BIG,
                                ALU.is_ge, ALU.mult)
        nc.vector.tensor_add(lx[:, :], lg[:, :], lx[:, :])
        nc.vector.tensor_reduce(q[:, 1:2], lx[:, :], AX.X, ALU.max)
        e2 = small.tile([BLOC, E], f32, tag="e2")
        nc.vector.tensor_scalar(e2[:, :], lx[:, :], q[:, 1:2], None, ALU.is_ge)
        w2 = small.tile([BLOC, 2], f32, tag="w2")
        nc.scalar.activation(w2[:, 0:1], q[:, 1:2], AF.Sigmoid,
                             bias=q[:, 0:1], scale=-1.0)  # sigmoid(m1*-1+... )
        # careful: want sigmoid(m2 - m1): in_=q[:,1:2], scale=1, bias=-m1
        gates = small.tile([BLOC, E], f32, tag="gates")
        w1 = small.tile([BLOC, 2], f32, tag="w1")
        nc.vector.tensor_scalar(w1[:, 0:1], w2[:, 0:1], -1.0, 1.0,
                                ALU.mult, ALU.add)
        # w2 tile holds sigmoid(m1-m2) = weight of the argmax (top-1)
        g1t = small.tile([BLOC, E], f32, tag="g1t")
        nc.vector.tensor_scalar_mul(g1t[:, :], e1[:, :], w2[:, 0:1])
        g2t = small.tile([BLOC, E], f32, tag="g2t")
        nc.vector.tensor_scalar_mul(g2t[:, :], e2[:, :], w1[:, 0:1])
        nc.vector.tensor_add(gates[:, :], g1t[:, :], g2t[:, :])
        nc.sync.dma_start(gates_d[:, :], gates[:, :])
        # broadcast gates to all partitions: gates -> dram [1,32] -> matmul
        nc.sync.dma_start(gsc[:, :], gates[:, :])
        g1x = small.tile([1, BLOC * E], f32, tag="g1x")
        nc.gpsimd.dma_start(g1x[:, :], gsc[:, :])
        gbp = pbc.tile([128, L], f32, tag="pb")
        nc.tensor.matmul(gbp[:, 0:BLOC * E], ones1[:], g1x[:, :], start=True, stop=True)
        gbc = work.tile([128, BLOC * E], f32, tag="gbc")
        nc.scalar.activation(gbc[:, :], gbp[:, 0:BLOC * E], AF.Copy)

        # ---------------- phase C: combined expert matmul ------------------
        for b in range(BLOC):
            x1, x2 = xt[b]
            cA = pbc.tile([128, L], f32, tag="pb")
            cB = pbc.tile([128, L], f32, tag="pb")
            gis = []
            for e in range(E):
                gi = work.tile([128, 128], f16, tag=f"gi{e}")
                nc.vector.tensor_scalar_mul(gi[:, :], eye16[:, :],
                                            gbc[:, E * b + e:E * b + e + 1])
                gis.append(gi)
            gis.append(eye16)
            for e in range(E + 1):
                nc.tensor.matmul(cA[:, :], gis[e][:, :], wstA[e][:],
                                 start=(e == 0), stop=(e == E))
            for e in range(E + 1):
                nc.tensor.matmul(cB[0:65, :], gis[e][0:65, 0:65], wstB[e][:],
                                 start=(e == 0), stop=(e == E))
            c16A = work.tile([128, L], f16, tag="c16A")
            c16B = work.tile([65, L], f16, tag="c16B")
            nc.scalar.activation(c16A[:, :], cA[:, :], AF.Copy)
            nc.scalar.activation(c16B[:, :], cB[0:65, :], AF.Copy)
            for mi, (m0, mw) in enumerate(((0, 128), (128, 64))):
                for ni, (nn0, nw) in enumerate(((0, 431), (431, 431))):
                    op = outp.tile([128, 431], f32, tag="op")
                    nc.tensor.matmul(op[:mw, :nw], c16A[:, m0:m0 + mw],
                                     x1[:, nn0:nn0 + nw], start=True, stop=False)
                    nc.tensor.matmul(op[:mw, :nw], c16B[:, m0:m0 + mw],
                                     x2[:, nn0:nn0 + nw], start=False, stop=True)
                    osb = work.tile([128, 431], f32, tag="osb")
                    nc.scalar.activation(osb[:mw, :nw], op[:mw, :nw], AF.Copy)
                    nc.sync.dma_start(out_d[b, m0:m0 + mw, nn0:nn0 + nw],
                                      osb[:mw, :nw])
    nc.finalize()
    return nc


def kernel(x, w_start, b_start, w_gate, W_exp, b_exp, W_end, b_end):
    global LAST_EXEC_NS
    x = np.asarray(x)
    consts = _host_consts(np.asarray(w_start), np.asarray(b_start),
                          np.asarray(w_gate), np.asarray(W_exp),
                          np.asarray(b_exp), np.asarray(W_end),
                          np.asarray(b_end))
    x3 = x[:, :, :, 0]
    in_maps = []
    for i in range(NCORES):
        xs = x3[i * BLOC:(i + 1) * BLOC]            # [8, L, N]
        x16 = np.ones((BLOC, L + 1, N), np.float16)
        x16[:, :L, :] = xs.astype(np.float16)
        m = {"x16": x16}
        m.update(consts)
        in_maps.append(m)
    nc = _build_graph()
    import os
    tr = os.environ.get("BASS_DO_TRACE", "1") == "1"
    try:
        res = run_bass_kernel_spmd(nc, in_maps, list(range(NCORES)), trace=tr)
    except Exception:
        if not tr:
            raise
        res = run_bass_kernel_spmd(nc, in_maps, list(range(NCORES)))
    LAST_EXEC_NS = res.exec_time_ns
    outs = np.concatenate([r["out"] for r in res.results], axis=0)
    gates = np.concatenate([r["gates"] for r in res.results], axis=0)
    # balance loss on host (fp32, mirrors reference)
    imp = gates.sum(axis=0)
    load = (gates > 0).sum(axis=0).astype(np.float32)

    def cv2(v):
        return v.var(ddof=1) / (v.mean() ** 2 + 1e-10)

    loss = np.float32(0.01 * (cv2(imp) + cv2(load)))
    return outs.reshape(B, L, N, C).astype(np.float32), loss
